# revision 7
# baseline (speedup 1.0000x reference)
"""AttentionBlock kernel for Trainium2, data-parallel over batch on 8 NeuronCores.

Per-core computation (one batch element, x_b: [256, 4096] = [C, H*W]):
  GroupNorm(8 groups) folded into the QKV projection:
    xn = x*scale_c + shift_c   (per-channel affine from group stats)
    qkv = W_qkv xn + b  ==  (W_qkv * scale_c) x + (W_qkv shift + b)
  q,k produced as [C, N] (channels on partitions);
  v produced directly transposed as vT [N, C] (tokens on partitions) so the
  second attention matmul needs no transposes.
  Attention: S'[m,n] = sum_c k[c,m] q[c,n] (S transposed, m on partitions),
  P' = exp(S'/16) (unsafe softmax: logits ~N(0,1), no max pass needed),
  out[c,n] = sum_m vT[m,c] P'[m,n] accumulated in PSUM over the 32 m-tiles,
  denominator d[n] = sum_m P'[m,n] via a ones-column matmul into one PSUM row.
  Normalization is deferred past the proj matmul (it commutes), keeping the
  softmax-denominator reciprocal off the PE critical path:
    y = (proj(out_unnorm) * (1/d)) + proj_b + x
  1/d is computed on all 128 partitions after a K=1 broadcast matmul.
All big matmuls run as float32r (full-rate fp32 on the PE array); f32r
operands must be produced by a rounding instruction or DMA-fed from an
f32r-declared DRAM parameter.
"""

import sys

sys.path.insert(0, "/opt/trn_rl_repo")

import numpy as np

import concourse.bass as bass  # noqa: F401
import concourse.mybir as mybir
import concourse.tile as tile
from concourse import bacc
from concourse.bass_utils import run_bass_kernel_spmd

F32 = mybir.dt.float32
F32R = mybir.dt.float32r
AF = mybir.ActivationFunctionType
ALU = mybir.AluOpType

C = 256
N = 4096
GROUPS = 8
EPS = 1e-5
CT = 2          # channel tiles of 128
MT = 32         # m (key/token) tiles of 128
NB = 8          # n (query/token) chunks of 512
NCHUNK = 512
SCALE = 1.0 / 16.0  # 1/sqrt(C)
GSIZE = C // GROUPS
GN_COUNT = float(GSIZE * N)
XCH = 2         # x DMA/stat chunks per c-tile
XCW = N // XCH  # 2048


def _build():
    nc = bacc.Bacc("TRN2", target_bir_lowering=False)

    x_d = nc.declare_dram_parameter("x", [C, N], F32R, isOutput=False)
    wqkvT_d = nc.declare_dram_parameter("wqkvT", [C, 3 * C], F32, isOutput=False)
    wpT_d = nc.declare_dram_parameter("wpT", [C, C], F32R, isOutput=False)
    bqk_d = nc.declare_dram_parameter("bqk", [128, 4], F32, isOutput=False)
    bvrow_d = nc.declare_dram_parameter("bvrow", [1, C], F32, isOutput=False)
    bp_d = nc.declare_dram_parameter("bp", [128, 2], F32, isOutput=False)
    gamma_d = nc.declare_dram_parameter("gamma", [128, 2], F32, isOutput=False)
    beta_d = nc.declare_dram_parameter("beta", [128, 2], F32, isOutput=False)
    sel_d = nc.declare_dram_parameter("sel", [128, 2 * GROUPS], F32, isOutput=False)
    selb_d = nc.declare_dram_parameter("selb", [GROUPS, C], F32, isOutput=False)
    ones_d = nc.declare_dram_parameter("ones", [128, 128], F32R, isOutput=False)
    out_d = nc.declare_dram_parameter("out", [C, N], F32, isOutput=True)

    with tile.TileContext(nc) as tc:
        with (
            tc.tile_pool(name="const", bufs=1) as cp,
            tc.tile_pool(name="work", bufs=1) as wp,
        ):
            # ---- x loads (chunked so GN stats overlap the DMA) ----
            xt = []
            for t in range(CT):
                xtile = cp.tile([128, N], F32R, name=f"x{t}", tag=f"x{t}")
                for ch in range(XCH):
                    nc.sync.dma_start(xtile[:, ch * XCW:(ch + 1) * XCW],
                                      x_d[t * 128:(t + 1) * 128, ch * XCW:(ch + 1) * XCW])
                xt.append(xtile)
            # ---- GN statistics per chunk: sx partials (DVE) | sxx partials (ACT) ----
            stats = []
            for t in range(CT):
                st = cp.tile([128, 2 * XCH], F32, name=f"stats{t}", tag=f"stats{t}")
                for ch in range(XCH):
                    xv = xt[t][:, ch * XCW:(ch + 1) * XCW].bitcast(F32)
                    nc.vector.tensor_reduce(st[:, ch:ch + 1], xv, mybir.AxisListType.X, ALU.add)
                    scratch = wp.tile([128, XCW], F32, tag="scratch", name="scratch")
                    nc.scalar.activation(scratch[:], xv, AF.Square,
                                         accum_out=st[:, XCH + ch:XCH + ch + 1])
                stats.append(st)

            # ---- remaining loads ----
            wT = []
            wpt = []
            for t in range(CT):
                wtile = cp.tile([128, 3 * C], F32, name=f"wT{t}", tag=f"wT{t}")
                nc.sync.dma_start(wtile[:], wqkvT_d[t * 128:(t + 1) * 128, :])
                wT.append(wtile)
                wptile = cp.tile([128, C], F32R, name=f"wpT{t}", tag=f"wpT{t}")
                nc.sync.dma_start(wptile[:], wpT_d[t * 128:(t + 1) * 128, :])
                wpt.append(wptile)
            bqk = cp.tile([128, 4], F32, name="bqk", tag="bqk")
            nc.sync.dma_start(bqk[:], bqk_d[:])
            bvrow = cp.tile([1, C], F32, name="bvrow", tag="bvrow")
            nc.sync.dma_start(bvrow[:], bvrow_d[:])
            bp = cp.tile([128, 2], F32, name="bp", tag="bp")
            nc.sync.dma_start(bp[:], bp_d[:])
            gamma = cp.tile([128, 2], F32, name="gamma", tag="gamma")
            nc.sync.dma_start(gamma[:], gamma_d[:])
            beta = cp.tile([128, 2], F32, name="beta", tag="beta")
            nc.sync.dma_start(beta[:], beta_d[:])
            sel = cp.tile([128, 2 * GROUPS], F32, name="sel", tag="sel")
            nc.sync.dma_start(sel[:], sel_d[:])
            selb = cp.tile([GROUPS, C], F32, name="selb", tag="selb")
            nc.sync.dma_start(selb[:], selb_d[:])
            ones = cp.tile([128, 128], F32R, name="ones", tag="ones")
            nc.sync.dma_start(ones[:], ones_d[:])

            # ---- setup-phase PSUM pool (closed before the attention loop) ----
            with tc.tile_pool(name="ps0", bufs=2, space="PSUM") as ps0:
                g_ps = ps0.tile([GROUPS, 2 * XCH], F32, tag="small", name="g_ps")
                nc.tensor.matmul(g_ps[:], sel[:, 0:GROUPS], stats[0][:], start=True, stop=False)
                nc.tensor.matmul(g_ps[:], sel[:, GROUPS:2 * GROUPS], stats[1][:], start=False, stop=True)
                # per-group mean / rstd on partitions 0..7
                g_mr = cp.tile([GROUPS, 2], F32, name="g_mr", tag="g_mr")
                gtmp = cp.tile([GROUPS, 5], F32, name="gtmp", tag="gtmp")
                g_sb = cp.tile([GROUPS, 2 * XCH], F32, name="g_sb", tag="g_sb")
                nc.scalar.copy(g_sb[:], g_ps[:])
                nc.vector.tensor_add(gtmp[:, 3:4], g_sb[:, 0:1], g_sb[:, 1:2])
                nc.vector.tensor_add(gtmp[:, 4:5], g_sb[:, 2:3], g_sb[:, 3:4])
                nc.vector.tensor_scalar_mul(g_mr[:, 0:1], gtmp[:, 3:4], 1.0 / GN_COUNT)
                nc.vector.tensor_scalar_mul(gtmp[:, 0:1], gtmp[:, 4:5], 1.0 / GN_COUNT)
                nc.vector.tensor_mul(gtmp[:, 1:2], g_mr[:, 0:1], g_mr[:, 0:1])
                nc.vector.tensor_sub(gtmp[:, 2:3], gtmp[:, 0:1], gtmp[:, 1:2])
                gvar = cp.tile([GROUPS, 1], F32, name="gvar", tag="gvar")
                nc.vector.tensor_scalar_add(gvar[:], gtmp[:, 2:3], EPS)
                gstd = cp.tile([GROUPS, 1], F32, name="gstd", tag="gstd")
                nc.scalar.activation(gstd[:], gvar[:], AF.Sqrt)
                nc.vector.reciprocal(g_mr[:, 1:2], gstd[:])

                # broadcast group mean/rstd to per-channel scale/shift
                scale_t = []
                shift_t = []
                for t in range(CT):
                    mr_ps = ps0.tile([128, 2], F32, tag="small", name="mr_ps")
                    nc.tensor.matmul(mr_ps[:], selb[:, t * 128:(t + 1) * 128], g_mr[:],
                                     start=True, stop=True)
                    mr = cp.tile([128, 2], F32, name=f"mr{t}", tag=f"mr{t}")
                    nc.scalar.copy(mr[:], mr_ps[:])
                    sc = cp.tile([128, 1], F32, name=f"scale{t}", tag=f"scale{t}")
                    nc.vector.tensor_mul(sc[:], mr[:, 1:2], gamma[:, t:t + 1])
                    tmp = cp.tile([128, 1], F32, name=f"mscale{t}", tag=f"mscale{t}")
                    nc.vector.tensor_mul(tmp[:], mr[:, 0:1], sc[:])
                    sh = cp.tile([128, 1], F32, name=f"shift{t}", tag=f"shift{t}")
                    nc.vector.tensor_sub(sh[:], beta[:, t:t + 1], tmp[:])
                    scale_t.append(sc)
                    shift_t.append(sh)

                # adjusted qkv weights: wadj[c, o] = wT[c, o] * scale_c
                wadj = []
                for t in range(CT):
                    wa = cp.tile([128, 3 * C], F32R, name=f"wadj{t}", tag=f"wadj{t}")
                    nc.vector.tensor_scalar_mul(wa[:], wT[t][:], scale_t[t][:])
                    wadj.append(wa)
                # q/k bias: btot[o] = qkv_b[o] + sum_c wT[c,o]*shift_c  (o in 0..512)
                bias_ps = ps0.tile([128, 4], F32, tag="small", name="bias_ps")
                for ot in range(4):
                    for t in range(CT):
                        nc.tensor.matmul(bias_ps[:, ot:ot + 1],
                                         wT[t][:, ot * 128:(ot + 1) * 128],
                                         shift_t[t][:],
                                         start=(t == 0), stop=(t == CT - 1))
                btot = cp.tile([128, 4], F32, name="btot", tag="btot")
                nc.vector.tensor_add(btot[:], bias_ps[:], bqk[:])
                # v bias row: bvtot[1, c] = qkv_b_v[c] + sum_i shift_i wvT[i, c]
                bv_ps = ps0.tile([1, C], F32, tag="small", name="bv_ps")
                for t in range(CT):
                    nc.tensor.matmul(bv_ps[:], shift_t[t][:], wT[t][:, 2 * C:3 * C],
                                     start=(t == 0), stop=(t == CT - 1))
                bvtot = cp.tile([1, C], F32R, name="bvtot", tag="bvtot")
                nc.vector.tensor_add(bvtot[:], bv_ps[:], bvrow[:])

            with tc.tile_pool(name="ps", bufs=1, space="PSUM") as ps:
                # ---- QKV projections (evictions alternate ACT / DVE) ----
                q_sb = [cp.tile([128, N], F32R, name=f"q{t}", tag=f"q{t}") for t in range(CT)]
                k_sb = [cp.tile([128, N], F32R, name=f"k{t}", tag=f"k{t}") for t in range(CT)]
                dests = [q_sb[0], q_sb[1], k_sb[0], k_sb[1]]
                for ot in range(4):
                    for mc in range(NB):
                        qk_ps = ps.tile([128, NCHUNK], F32, tag="s", bufs=3, name="qk_ps")
                        for t in range(CT):
                            nc.tensor.matmul(qk_ps[:],
                                             wadj[t][:, ot * 128:(ot + 1) * 128],
                                             xt[t][:, mc * NCHUNK:(mc + 1) * NCHUNK],
                                             start=(t == 0), stop=(t == CT - 1))
                        dst = dests[ot][:, mc * NCHUNK:(mc + 1) * NCHUNK]
                        if mc % 2 == 0:
                            nc.scalar.activation(dst, qk_ps[:], AF.Identity,
                                                 bias=btot[:, ot:ot + 1])
                        else:
                            nc.vector.tensor_scalar_add(dst, qk_ps[:], btot[:, ot:ot + 1])
                # vT[m, c]: vT = x^T wadj_v + ones x bvtot
                vT = cp.tile([128, MT * C], F32R, name="vT", tag="vT")
                for mt in range(MT):
                    vt_ps = ps.tile([128, C], F32, tag="s", bufs=3, name="vt_ps")
                    for t in range(CT):
                        nc.tensor.matmul(vt_ps[:],
                                         xt[t][:, mt * 128:(mt + 1) * 128],
                                         wadj[t][:, 2 * C:3 * C],
                                         start=(t == 0), stop=False)
                    nc.tensor.matmul(vt_ps[:], ones[0:1, :], bvtot[:],
                                     start=False, stop=True)
                    dst = vT[:, mt * C:(mt + 1) * C]
                    if mt % 2 == 0:
                        nc.scalar.copy(dst, vt_ps[:])
                    else:
                        nc.vector.tensor_copy(dst, vt_ps[:])

                # ---- attention ----
                for nb in range(NB):
                    nsl = slice(nb * NCHUNK, (nb + 1) * NCHUNK)
                    out_ps = [ps.tile([128, NCHUNK], F32, tag="out", bufs=4, name=f"outp{_t}")
                              for _t in range(CT)]
                    d_ps = ps.tile([1, NCHUNK], F32, tag="d", bufs=1, name="d_ps")
                    for mb in range(MT):
                        s_ps = ps.tile([128, NCHUNK], F32, tag="s", bufs=3, name="s_ps")
                        for t in range(CT):
                            nc.tensor.matmul(s_ps[:],
                                             k_sb[t][:, mb * 128:(mb + 1) * 128],
                                             q_sb[t][:, nsl],
                                             start=(t == 0), stop=(t == CT - 1))
                        p_sb = wp.tile([128, NCHUNK], F32R, tag="p", bufs=4, name="p_sb")
                        nc.scalar.activation(p_sb[:], s_ps[:], AF.Exp, scale=SCALE)
                        first, last = (mb == 0), (mb == MT - 1)
                        for t in range(CT):
                            nc.tensor.matmul(out_ps[t][:],
                                             vT[:, mb * C + t * 128: mb * C + (t + 1) * 128],
                                             p_sb[:], start=first, stop=last)
                        nc.tensor.matmul(d_ps[:], ones[:, 0:1], p_sb[:],
                                         start=first, stop=last)
                    # evict attention numerator (unnormalized) to SBUF
                    att = []
                    for t in range(CT):
                        at = wp.tile([128, NCHUNK], F32R, tag="att", bufs=3, name="att")
                        if t == 0:
                            nc.scalar.copy(at[:], out_ps[t][:])
                        else:
                            nc.vector.tensor_copy(at[:], out_ps[t][:])
                        att.append(at)
                    # denominator path (off the PE critical path)
                    d_sb = wp.tile([1, NCHUNK], F32R, tag="dsb", bufs=2, name="d_sb")
                    nc.scalar.copy(d_sb[:], d_ps[:])
                    rdb_ps = ps.tile([128, NCHUNK], F32, tag="s", bufs=3, name="rdb_ps")
                    nc.tensor.matmul(rdb_ps[:], ones[0:1, :], d_sb[:], start=True, stop=True)
                    rdb = wp.tile([128, NCHUNK], F32, tag="rdb", bufs=2, name="rdb")
                    nc.vector.reciprocal(rdb[:], rdb_ps[:])
                    # proj into freed "out" slots, then normalize + bias + residual
                    for ot in range(CT):
                        z_ps = ps.tile([128, NCHUNK], F32, tag="out", bufs=4, name="z_ps")
                        for t in range(CT):
                            nc.tensor.matmul(z_ps[:],
                                             wpt[t][:, ot * 128:(ot + 1) * 128],
                                             att[t][:],
                                             start=(t == 0), stop=(t == CT - 1))
                        y = wp.tile([128, NCHUNK], F32, tag="y", bufs=3, name="y")
                        nc.vector.tensor_mul(y[:], z_ps[:], rdb[:])
                        nc.vector.scalar_tensor_tensor(
                            y[:], in0=y[:], scalar=bp[:, ot:ot + 1],
                            in1=xt[ot][:, nsl].bitcast(F32), op0=ALU.add, op1=ALU.add)
                        nc.sync.dma_start(out_d[ot * 128:(ot + 1) * 128, nsl], y[:])
    nc.compile()
    return nc


_NC = None


def _get_nc():
    global _NC
    if _NC is None:
        _NC = _build()
    return _NC


def kernel(x, gn_w, gn_b, qkv_w, qkv_b, proj_w, proj_b):
    x = np.asarray(x, dtype=np.float32)
    b = x.shape[0]
    assert b == 8 and x.shape[1] == C
    xs = x.reshape(b, C, N)

    wqkvT = np.ascontiguousarray(np.asarray(qkv_w, np.float32).T)      # [C, 3C]
    wpT = np.ascontiguousarray(np.asarray(proj_w, np.float32).T)       # [C, C]
    qkv_b = np.asarray(qkv_b, np.float32)
    bqk = np.ascontiguousarray(qkv_b[:2 * C].reshape(4, 128).T)        # [128, 4]
    bvrow = np.ascontiguousarray(qkv_b[2 * C:].reshape(1, C))          # [1, C]
    bp = np.ascontiguousarray(np.asarray(proj_b, np.float32).reshape(CT, 128).T)
    gamma = np.ascontiguousarray(np.asarray(gn_w, np.float32).reshape(CT, 128).T)
    beta = np.ascontiguousarray(np.asarray(gn_b, np.float32).reshape(CT, 128).T)

    # group selectors: channel c -> group c // GSIZE
    sel = np.zeros((128, 2 * GROUPS), np.float32)
    selb = np.zeros((GROUPS, C), np.float32)
    for t in range(CT):
        for p in range(128):
            g = (t * 128 + p) // GSIZE
            sel[p, t * GROUPS + g] = 1.0
            selb[g, t * 128 + p] = 1.0

    nc = _get_nc()
    shared = {
        "wqkvT": wqkvT, "wpT": wpT, "bqk": bqk, "bvrow": bvrow, "bp": bp,
        "gamma": gamma, "beta": beta, "sel": sel, "selb": selb,
        "ones": np.ones((128, 128), np.float32),
    }
    in_maps = [dict(shared, x=np.ascontiguousarray(xs[i])) for i in range(b)]
    res = run_bass_kernel_spmd(nc, in_maps, core_ids=list(range(8)))
    out = np.stack([res.results[i]["out"] for i in range(b)])
    return out.reshape(x.shape).astype(np.float32)
